# revision 1
# baseline (speedup 1.0000x reference)
"""Trainium2 Bass kernel for causal multi-head attention (B=4, T=2048, D=1024, H=16).

Sharding: tensor-parallel over heads. Each of the 8 NeuronCores owns 2 heads:
it computes Q/K/V projections for its head-slice over all tokens, runs causal
attention, then an AllToAll re-shards the attention output from head-sharded
to token-sharded so each core computes the final FC layer for its 1024-token
block with the full weight matrix. No reduction collective is needed.

All matmuls run as float32r (TF32-like, full PE rate at moving dim >= 256);
everything else stays fp32. Scores are computed transposed (S^T = K Q^T) so
softmax normalization lands on the PV matmul's free dim; the denominator is
obtained by augmenting V with a ones column, and its reciprocal is broadcast
across partitions with a selector matmul (partition-shifted DVE/DMA ops are
avoided entirely -- they were observed to misbehave on this stack).
"""
import sys

for _p in ("/opt/trn_rl_repo",):
    if _p not in sys.path:
        sys.path.insert(0, _p)

import numpy as np

import concourse.bass as bass
import concourse.mybir as mybir
import concourse.tile as tile
from concourse import bacc
from concourse.bass_utils import run_bass_kernel_spmd

f32 = mybir.dt.float32
f32r = mybir.dt.float32r
bf16 = mybir.dt.bfloat16
EXP = mybir.ActivationFunctionType.Exp

B, T, D, H, HD = 4, 2048, 1024, 16, 64
NCORES = 8
HPC = H // NCORES          # heads per core = 2
BT = B * T                 # 8192
CH = 512                   # token chunk (matmul moving dim)
NCH_B = T // CH            # 4 projection chunks per batch
QC = T // CH               # 4 query chunks per batch
NKV_B = T // 128           # 16 kv tiles of 128 per batch
ROWS = BT // NCORES        # 1024 output token rows per core
SCALE = 1.0 / 8.0          # 1/sqrt(HD)

_CACHE = {}


def _build(sim=False, no_collective=False, reps=1, n_ag=1, phases=('qkv','attn','fc'), dup=()):
    nc = bacc.Bacc("TRN2", target_bir_lowering=False, debug=False,
                   num_devices=1 if sim else NCORES)

    xT = nc.dram_tensor("xT", [D, BT], bf16, kind="ExternalInput").ap()
    wqkv = nc.dram_tensor("wqkv", [D, 3 * 128], bf16, kind="ExternalInput").ap()
    bqkv = nc.dram_tensor("bqkv", [1, 3 * 128], f32, kind="ExternalInput").ap()
    wfc_d = nc.dram_tensor("wfc", [D, 128], f32, kind="ExternalInput").ap()
    bfc_d = nc.dram_tensor("bfc", [1, 128], f32, kind="ExternalInput").ap()
    hm_d = nc.dram_tensor("hm", [128, 896], f32, kind="ExternalInput").ap()
    zl_d = nc.dram_tensor("zl", [65, 64], f32, kind="ExternalInput").ap()
    id_d = nc.dram_tensor("ident", [128, 128], f32, kind="ExternalInput").ap()
    ones_d = nc.dram_tensor("ones", [128, CH], f32, kind="ExternalInput").ap()
    zer_d = nc.dram_tensor("zer", [65, CH], f32, kind="ExternalInput").ap()
    outT = nc.dram_tensor("outT", [128, BT], f32, kind="ExternalOutput").ap()

    with tile.TileContext(nc) as tc:
        with tc.tile_pool(name="const", bufs=1) as cst, \
             tc.tile_pool(name="dram", bufs=1, space="DRAM") as dpool:

            # ---- constants (host-provided) ----
            ones_r = cst.tile([128, CH], f32r)
            nc.sync.dma_start(ones_r[:], ones_d[:].bitcast(f32r))
            hm = cst.tile([128, 896], f32)
            nc.sync.dma_start(hm[:], hm_d[:])
            zl = cst.tile([65, 64], f32r)       # selector: row 64 = 1, rest 0
            nc.sync.dma_start(zl[:], zl_d[:].bitcast(f32r))
            bias_q = cst.tile([1, 3 * 128], f32r)
            nc.sync.dma_start(bias_q[:], bqkv[:].bitcast(f32r))
            bias_f = cst.tile([1, 128], f32r)
            nc.sync.dma_start(bias_f[:], bfc_d[:].bitcast(f32r))
            # reciprocal staging tiles (rows 0..63 stay zero forever)
            rc = []
            for h in range(HPC):
                t = cst.tile([65, CH], f32r, name=f"recip{h}")
                nc.sync.dma_start(t[:], zer_d[:].bitcast(f32r))
                rc.append(t)
            rtmp = cst.tile([65, CH], f32)      # fp32 reciprocal staging
            ones_b = cst.tile([1, CH], bf16)
            nc.vector.tensor_copy(ones_b[:], ones_r[0:1, :].bitcast(f32))
            biasb = cst.tile([1, 3 * 128], bf16)
            nc.vector.tensor_copy(biasb[:], bias_q[:].bitcast(f32))

            # attention output, head-sharded: per local head [64, BT]
            attn = [cst.tile([64, BT], f32r, name=f"attn{h}")
                    for h in range(HPC)]

            # qkv weights: 8 d-tiles of [128, 384] = [q128 | k128 | v128]
            wq = cst.tile([128, 8 * 384], bf16)
            for d in range(8):
                nc.sync.dma_start(wq[:, d * 384:(d + 1) * 384],
                                  wqkv[d * 128:(d + 1) * 128, :])

            for _rep in range(reps):
                # ---- per-batch QKV projection + attention ----
                with tc.tile_pool(name="work", bufs=1) as wk, \
                     tc.tile_pool(name="ps", bufs=1, space="PSUM") as ps:
                    for b in range(B):
                        t0 = b * T
                        qt = wk.tile([128, T], f32r, tag="qt", bufs=2, name=f"qt{b}")
                        kt = wk.tile([128, T], f32r, tag="kt", bufs=2, name=f"kt{b}")
                        vsb = wk.tile([128, NKV_B * 130], f32r, tag="vsb", bufs=2,
                                      name=f"vsb{b}")
                        # ones columns (denominator) for all 16 kv tiles of batch b
                        v3 = vsb.rearrange("p (t c) -> p t c", c=130)
                        src1 = ones_d[:, 0:NKV_B].rearrange("p (t c) -> p t c", c=1)
                        nc.sync.dma_start(v3[:, :, 64:65], src1.bitcast(f32r))
                        nc.sync.dma_start(v3[:, :, 129:130], src1.bitcast(f32r))

                        for ch in [c for c in range(NCH_B) for _ in range(2 if 'qkv' in dup else 1)]:
                            c0 = t0 + ch * CH
                            xt = wk.tile([128, 8 * CH], bf16, tag="xt", bufs=2,
                                         name=f"xt{b}_{ch}")
                            xt3 = xt.rearrange("p (d c) -> p d c", d=8)
                            xs3 = xT[:, c0:c0 + CH].rearrange(
                                "(d p) c -> p d c", p=128)
                            nc.sync.dma_start(xt3[:], xs3)
                            cs = ch * CH
                            # Q^T chunk
                            psq = ps.tile([128, CH], f32, tag="mm", bufs=2,
                                          name=f"psq{b}_{ch}")
                            for d in range(8):
                                nc.tensor.matmul(psq[:],
                                                 wq[:, d * 384:d * 384 + 128],
                                                 xt[:, d * CH:(d + 1) * CH],
                                                 start=(d == 0), stop=False)
                            nc.tensor.matmul(psq[:], bias_q[0:1, 0:128],
                                             ones_r[0:1, :], start=False, stop=True)
                            nc.vector.tensor_copy(qt[:, cs:cs + CH], psq[:])
                            # K^T chunk
                            psk = ps.tile([128, CH], f32, tag="mm", bufs=2,
                                          name=f"psk{b}_{ch}")
                            for d in range(8):
                                nc.tensor.matmul(psk[:],
                                                 wq[:, d * 384 + 128:d * 384 + 256],
                                                 xt[:, d * CH:(d + 1) * CH],
                                                 start=(d == 0), stop=False)
                            nc.tensor.matmul(psk[:], bias_q[0:1, 128:256],
                                             ones_r[0:1, :], start=False, stop=True)
                            nc.vector.tensor_copy(kt[:, cs:cs + CH], psk[:])
                            # V directly token-major: lhsT = x tile, rhs = W_v
                            for sb in range(CH // 128):
                                kvt = ch * 4 + sb   # kv tile idx within batch
                                psv = ps.tile([128, 128], f32, tag="mm", bufs=2,
                                              name=f"psv{b}_{ch}_{sb}")
                                for d in range(8):
                                    nc.tensor.matmul(
                                        psv[:],
                                        xt3[:, d, sb * 128:(sb + 1) * 128],
                                        wq[:, d * 384 + 256:d * 384 + 384],
                                        start=(d == 0), stop=False)
                                nc.tensor.matmul(psv[:], ones_b[0:1, 0:128],
                                                 biasb[0:1, 256:384],
                                                 start=False, stop=True)
                                base = kvt * 130
                                nc.vector.tensor_copy(vsb[:, base:base + 64],
                                                      psv[:, 0:64])
                                nc.vector.tensor_copy(vsb[:, base + 65:base + 129],
                                                      psv[:, 64:128])

                        # ---- causal attention for batch b ----
                        for qc in range(QC):
                            g0 = t0 + qc * CH
                            nkv = 4 * (qc + 1)
                            pv = [ps.tile([128, CH], f32, tag=f"pv{h}", bufs=1,
                                          name=f"pv{h}_{b}_{qc}")
                                  for h in range(HPC)]
                            for ki in range(nkv):
                                diag = ki - 4 * qc  # >=0 on diagonal block tiles
                                st = ps.tile([128, 2 * CH], f32, tag="s", bufs=2,
                                             name=f"s_{b}_{qc}_{ki}")
                                pt = wk.tile([128, 2 * CH], f32r, tag="p", bufs=3,
                                             name=f"p_{b}_{qc}_{ki}")
                                for h in range(HPC):
                                    nc.tensor.matmul(
                                        st[:, h * CH:(h + 1) * CH],
                                        kt[64 * h:64 * h + 64,
                                           ki * 128:(ki + 1) * 128],
                                        qt[64 * h:64 * h + 64,
                                           qc * CH:(qc + 1) * CH],
                                        start=True, stop=True,
                                        tile_position=(64 * h, 0))
                                nc.scalar.activation(pt[:], st[:], EXP,
                                                     scale=SCALE)
                                if diag >= 0:
                                    off = 384 - 128 * diag
                                    for h in range(HPC):
                                        nc.vector.tensor_mul(
                                            pt[:, h * CH:(h + 1) * CH],
                                            pt[:, h * CH:(h + 1) * CH],
                                            hm[:, off:off + CH])
                                for h in range(HPC):
                                    vb = ki * 130 + 65 * h
                                    nc.tensor.matmul(pv[h][0:65, :],
                                                     vsb[:, vb:vb + 65],
                                                     pt[:, h * CH:(h + 1) * CH],
                                                     start=(ki == 0),
                                                     stop=(ki == nkv - 1))
                            # normalize: reciprocal of denom row, broadcast, mul
                            for h in range(HPC):
                                nc.vector.reciprocal(rtmp[64:65, :],
                                                     pv[h][64:65, :])
                                nc.vector.tensor_copy(rc[h][64:65, :],
                                                      rtmp[64:65, :])
                                bc = ps.tile([64, CH], f32, tag="mm", bufs=2,
                                             name=f"bc{h}_{b}_{qc}")
                                nc.tensor.matmul(bc[:], zl[:], rc[h][:],
                                                 start=True, stop=True)
                                rb = wk.tile([64, CH], f32, tag="rb", bufs=2,
                                             name=f"rb{h}_{b}_{qc}")
                                nc.vector.tensor_copy(rb[:], bc[:])
                                nc.vector.tensor_mul(attn[h][:, g0:g0 + CH],
                                                     pv[h][0:64, :], rb[:])

                # ---- per-batch AllGather (overlaps later batches) ----
                ag_outs = []
                for b in range(B):
                    t0 = b * T
                    ag_in = dpool.tile([128, T], f32, name=f"ag_in{b}")
                    ag_out = dpool.tile([NCORES * 128, T], f32,
                                        name=f"ag_out{b}")
                    for h in range(HPC):
                        nc.sync.dma_start(
                            ag_in[64 * h:64 * h + 64, :],
                            attn[h][:, t0:t0 + T].bitcast(f32))
                    if sim or no_collective:
                        nc.sync.dma_start(ag_out[0:128, :], ag_in[:])
                    else:
                        for _agi in range(n_ag):
                            nc.gpsimd.collective_compute(
                                "AllGather", mybir.AluOpType.bypass,
                                replica_groups=[list(range(NCORES))],
                                ins=[ag_in.opt()], outs=[ag_out.opt()])
                    ag_outs.append(ag_out)

                # ---- final FC: this core computes its 128 output features for
                # all tokens (weight slice is per-core host input) ----
                with tc.tile_pool(name="fcp", bufs=1) as fcp, \
                     tc.tile_pool(name="psc", bufs=1, space="PSUM") as psc:
                    wfc = fcp.tile([128, 8 * 128], f32r)
                    for d in range(8):
                        nc.sync.dma_start(
                            wfc[:, d * 128:(d + 1) * 128],
                            wfc_d[d * 128:(d + 1) * 128, :].bitcast(f32r))
                    for oc in [o for o in range(BT // CH if 'fc' in phases else 0) for _ in range(2 if 'fc' in dup else 1)]:
                        fci = fcp.tile([128, 8 * CH], f32r, tag="fci", bufs=3,
                                       name=f"fci{oc}")
                        agb = ag_outs[oc // QC]
                        lc = oc % QC
                        for d in range(8):
                            nc.sync.dma_start(
                                fci[:, d * CH:(d + 1) * CH],
                                agb[d * 128:(d + 1) * 128,
                                    lc * CH:(lc + 1) * CH].bitcast(f32r))
                        pfc = psc.tile([128, CH], f32, tag="fc", bufs=4,
                                       name=f"pfc{oc}")
                        for d in range(8):
                            nc.tensor.matmul(pfc[:],
                                             wfc[:, d * 128:(d + 1) * 128],
                                             fci[:, d * CH:(d + 1) * CH],
                                             start=(d == 0), stop=False)
                        nc.tensor.matmul(pfc[:], bias_f[0:1, :],
                                         ones_r[0:1, :], start=False, stop=True)
                        ost = fcp.tile([128, CH], f32, tag="ost", bufs=4,
                                       name=f"ost{oc}")
                        nc.vector.tensor_copy(ost[:], pfc[:])
                        nc.sync.dma_start(outT[:, oc * CH:(oc + 1) * CH], ost[:])

    nc.compile()
    return nc


def _host_inputs(x, W_qkv, b_qkv, W_fc, b_fc):
    import ml_dtypes
    x = np.asarray(x, dtype=np.float32)
    W_qkv = np.asarray(W_qkv, dtype=np.float32)
    b_qkv = np.asarray(b_qkv, dtype=np.float32)
    W_fc = np.asarray(W_fc, dtype=np.float32)
    b_fc = np.asarray(b_fc, dtype=np.float32)

    xT = np.ascontiguousarray(x.reshape(BT, D).T).astype(ml_dtypes.bfloat16)
    hm = (np.arange(128)[:, None]
          <= np.arange(896)[None, :] - 384).astype(np.float32)
    zl = np.zeros((65, 64), np.float32)
    zl[64, :] = 1.0
    ident = np.eye(128, dtype=np.float32)
    ones = np.ones((128, CH), np.float32)
    zer = np.zeros((65, CH), np.float32)
    in_maps = []
    for c in range(NCORES):
        f0 = c * (HPC * HD)  # 128*c
        wfc_c = np.ascontiguousarray(W_fc[:, f0:f0 + 128])
        bfc_c = np.ascontiguousarray(b_fc[None, f0:f0 + 128])
        wq_c = np.ascontiguousarray(np.concatenate(
            [W_qkv[:, p * D + f0: p * D + f0 + 128] for p in range(3)],
            axis=1).astype(ml_dtypes.bfloat16))
        bq_c = np.ascontiguousarray(np.concatenate(
            [b_qkv[p * D + f0: p * D + f0 + 128] for p in range(3)])[None, :])
        in_maps.append({
            "xT": xT, "wqkv": wq_c, "bqkv": bq_c, "wfc": wfc_c, "bfc": bfc_c,
            "hm": hm, "zl": zl, "ident": ident, "ones": ones, "zer": zer,
        })
    return in_maps


def _get_nc():
    if "nc" not in _CACHE:
        _CACHE["nc"] = _build()
    return _CACHE["nc"]


def _assemble(results):
    blocks = [results[c]["outT"] for c in range(NCORES)]
    full = np.concatenate(blocks, axis=0)          # [D, BT], feature-major
    return np.ascontiguousarray(full.T).reshape(B, T, D).astype(np.float32)


def kernel(x, W_qkv, b_qkv, W_fc, b_fc):
    nc = _get_nc()
    in_maps = _host_inputs(x, W_qkv, b_qkv, W_fc, b_fc)
    res = run_bass_kernel_spmd(nc, in_maps, list(range(NCORES)))
    return _assemble(res.results)



# revision 2
# speedup vs baseline: 1.9033x; 1.9033x over previous
"""Trainium2 Bass kernel for causal multi-head attention (B=4, T=2048, D=1024, H=16).

Sharding: data-parallel, ZERO collectives. Each of the 8 cores owns half the
query tokens of one batch (core c -> batch c//2, variant c%2). Variant A takes
query blocks {0,3} of four 512-token blocks, variant B takes {1,2}; both
cover 20 real kv-tiles so the load is balanced. Every core computes the full
K/V projection for its batch locally (duplicated work is cheaper than any
cross-core communication), attention for all 16 heads over its 1024 queries,
and the final FC for its tokens with the full weight matrix.

The program is identical on all cores (SPMD); per-core asymmetry (query
positions, causal masks, padding) lives entirely in input DATA:
 - xq: the core's query tokens, pre-sliced on host
 - masks: per-core causal/padding mask tiles (mask index == kv tile index)
 - attention runs over a padded uniform shape: 8 kv-tiles for the "lo"
   query block, 16 for "hi"; out-of-range tiles are zeroed by all-zero masks.

Schedule: [ch0,ch1 KV + Q proj] -> [attn-lo || ch2,ch3 KV] ->
[attn-hi || FC-lo] -> [FC-hi]. The Act engine (exp) is the attention
bottleneck, so projection/FC matmuls fill the PE gaps; ch2/ch3 PSUM
drains go through DVE (bias on PE) to keep the Act queue exp-only.

All matmuls are bf16 (full PE rate); scores are computed transposed
(S^T = K Q^T) with the softmax denominator obtained from a ones-column
augmentation of V; the reciprocal is broadcast across partitions with a
selector matmul (partition-shifted DVE/DMA ops avoided). Q/K(ch01)/FC
biases are folded into the PSUM->SBUF copies on the Activation engine.
"""
import sys

for _p in ("/opt/trn_rl_repo",):
    if _p not in sys.path:
        sys.path.insert(0, _p)

import numpy as np

import concourse.bass as bass
import concourse.mybir as mybir
import concourse.tile as tile
from concourse import bacc
from concourse.bass_utils import run_bass_kernel_spmd

f32 = mybir.dt.float32
f32r = mybir.dt.float32r
bf16 = mybir.dt.bfloat16
f8 = mybir.dt.float8e4
DR = mybir.MatmulPerfMode.DoubleRow
WSCALE = 32.0
EXP = mybir.ActivationFunctionType.Exp
IDN = mybir.ActivationFunctionType.Identity

B, T, D, H, HD = 4, 2048, 1024, 16, 64
NCORES = 8
CH = 512                   # token chunk (matmul moving dim)
NCH = T // CH              # 4 kv token chunks per batch
NKV = T // 128             # 16 kv tiles of 128
QLOC = 2 * CH              # local query tokens per core (2 blocks of 512)
SCALE = 1.0 / 8.0          # 1/sqrt(HD)
NKLO, NKHI = 8, 16         # padded kv tiles for lo/hi query blocks

_CACHE = {}


def _build(sim=False, reps=1):
    nc = bacc.Bacc("TRN2", target_bir_lowering=False, debug=False,
                   num_devices=1 if sim else NCORES)

    xT_d = nc.dram_tensor("xT", [D, T], bf16, kind="ExternalInput").ap()
    xq_d = nc.dram_tensor("xq", [D, QLOC], bf16, kind="ExternalInput").ap()
    wqkv_d = nc.dram_tensor("wqkv", [D, 3 * D], bf16, kind="ExternalInput").ap()
    bq_d = nc.dram_tensor("bq", [128, 8], f32, kind="ExternalInput").ap()
    bk_d = nc.dram_tensor("bk", [128, 8], f32, kind="ExternalInput").ap()
    bkb_d = nc.dram_tensor("bkb", [1, D], bf16, kind="ExternalInput").ap()
    bv_d = nc.dram_tensor("bv", [1, D], bf16, kind="ExternalInput").ap()
    wfc_d = nc.dram_tensor("wfc", [128, 8 * 8 * 128], bf16,
                           kind="ExternalInput").ap()
    bfc_d = nc.dram_tensor("bfc", [128, 8], f32, kind="ExternalInput").ap()
    mask_d = nc.dram_tensor("masks", [128, NKHI * CH], bf16,
                            kind="ExternalInput").ap()
    zl_d = nc.dram_tensor("zl", [65, 64], f32, kind="ExternalInput").ap()
    id_d = nc.dram_tensor("id64", [64, 64], bf16, kind="ExternalInput").ap()
    outT_d = nc.dram_tensor("outT", [D, QLOC], f32, kind="ExternalOutput").ap()

    with tile.TileContext(nc) as tc:
        with tc.tile_pool(name="const", bufs=1) as cst:
            # ---- constants / persistent state ----
            zl = cst.tile([65, 64], f32r)        # selector: row 64 = 1
            nc.gpsimd.dma_start(zl[:], zl_d[:].bitcast(f32r))
            id64 = cst.tile([64, 64], bf16)      # identity: partition mover
            nc.gpsimd.dma_start(id64[:], id_d[:])
            bias_q = cst.tile([128, 8], f32)
            nc.gpsimd.dma_start(bias_q[:], bq_d[:])
            bias_k = cst.tile([128, 8], f32)
            nc.gpsimd.dma_start(bias_k[:], bk_d[:])
            bias_kb = cst.tile([1, D], bf16)     # k bias as row (PE matmul)
            nc.gpsimd.dma_start(bias_kb[:], bkb_d[:])
            bias_f = cst.tile([128, 8], f32)
            nc.gpsimd.dma_start(bias_f[:], bfc_d[:])
            bias_v = cst.tile([1, D], bf16)
            nc.gpsimd.dma_start(bias_v[:], bv_d[:])
            ones_r = cst.tile([1, CH], bf16)
            nc.vector.memset(ones_r[:], 1.0)

            # reciprocal staging (rows 0..63 stay zero forever; the
            # selector matmul multiplies them by zero, so they only
            # need to be finite)
            rtmp = []
            for e in range(2):
                t = cst.tile([65, CH], f32r, name=f"rtmp{e}")
                nc.vector.memset(t[:].bitcast(f32), 0.0)
                rtmp.append(t)

            # big persistent tensors
            kt = cst.tile([128, 8, T], bf16, name="kt")       # K^T by ftile
            qt = cst.tile([128, 8, QLOC], bf16, name="qt")    # Q^T by ftile
            vsb = cst.tile([128, NKV, H, 65], bf16, name="vsb")
            nc.vector.memset(vsb[:, :, :, 64:65], 1.0)        # denom ones col
            af = cst.tile([128, 8, QLOC], bf16, name="af")  # attn out, head
                                                  # pair split 0-63/64-127

            env = dict(
                xT=xT_d, xq=xq_d, wqkv=wqkv_d, wfc=wfc_d, mask_d=mask_d,
                outT=outT_d, bias_q=bias_q, bias_k=bias_k, bias_kb=bias_kb,
                bias_f=bias_f, bias_v=bias_v, ones_r=ones_r,
                zl=zl, rtmp=rtmp, id64=id64,
                kt=kt, qt=qt, vsb=vsb, af=af)
            for _rep in range(reps):
                _emit_body(nc, tc, env, rep=_rep)

    nc.compile()
    return nc


def _load_x_chunk(nc, wk, env, ch, rep):
    xt = wk.tile([128, 8, CH], bf16, tag="xt", bufs=2, name=f"xt{ch}_{rep}")
    nc.sync.dma_start(xt[:], env["xT"][:, ch * CH:(ch + 1) * CH].rearrange(
        "(d p) c -> p d c", p=128))
    return xt


def _k_ftile(nc, ps, env, wkv, xt, ch, f, rep, act_copy):
    kt = env["kt"]
    psk = ps.tile([128, CH], f32, tag="mm", bufs=2, name=f"psk{ch}_{f}_{rep}")
    for d in range(8):
        nc.tensor.matmul(psk[:], wkv[:, d, f * 128:(f + 1) * 128],
                         xt[:, d, :], start=(d == 0),
                         stop=(act_copy and d == 7))
    dst = kt[:, f, ch * CH:(ch + 1) * CH]
    if act_copy:
        nc.scalar.activation(dst, psk[:], IDN,
                             bias=env["bias_k"][:, f:f + 1])
    else:
        # keep Act exp-only: bias via PE row matmul, drain on DVE
        nc.tensor.matmul(psk[:],
                         env["bias_kb"][:, f * 128:(f + 1) * 128],
                         env["ones_r"][:],
                         start=False, stop=True)
        nc.vector.tensor_copy(dst, psk[:])


def _v_half(nc, ps, env, wkv, xt, ch, j, vh, rep):
    kvt = ch * 4 + j
    psv = ps.tile([128, CH], f32, tag="mm", bufs=2,
                  name=f"psv{ch}_{j}_{vh}_{rep}")
    for d in range(8):
        nc.tensor.matmul(
            psv[:], xt[:, d, j * 128:(j + 1) * 128],
            wkv[:, d, D + vh * CH:D + (vh + 1) * CH],
            start=(d == 0), stop=False)
    nc.tensor.matmul(psv[:], env["ones_r"][:, 0:128],
                     env["bias_v"][:, vh * CH:(vh + 1) * CH],
                     start=False, stop=True)
    # strided copy into [heads, 64] slots (8 heads per half)
    nc.vector.tensor_copy(
        env["vsb"][:, kvt, vh * 8:(vh + 1) * 8, 0:64],
        psv[:].rearrange("p (h v) -> p h v", h=8))


def _kv_chunk_units(nc, ps, env, wkv, ch, rep, act_copy, xt):
    units = [lambda f=f: _k_ftile(nc, ps, env, wkv, xt, ch, f, rep, act_copy)
             for f in range(8)]
    units += [lambda j=j, vh=vh: _v_half(nc, ps, env, wkv, xt, ch, j, vh, rep)
              for j in range(4) for vh in range(2)]
    return units


def _emit_kv_chunk(nc, wk, ps, env, wkv, ch, rep, act_copy, xt=None):
    """K^T + V projection for token chunk ch (512 tokens)."""
    if xt is None:
        xt = _load_x_chunk(nc, wk, env, ch, rep)
    for u in _kv_chunk_units(nc, ps, env, wkv, ch, rep, act_copy, xt):
        u()


def _emit_q_chunks(nc, wk, ps, env, wqq, rep):
    qt = env["qt"]
    for qs in range(2):
        xt = wk.tile([128, 8, CH], bf16, tag="xt", bufs=2, name=f"xq{qs}_{rep}")
        nc.sync.dma_start(xt[:], env["xq"][:, qs * CH:(qs + 1) * CH].rearrange(
            "(d p) c -> p d c", p=128))
        for f in range(8):
            psq = ps.tile([128, CH], f32, tag="mm", bufs=2,
                          name=f"psq{qs}_{f}_{rep}")
            for d in range(8):
                nc.tensor.matmul(psq[:], wqq[:, d, f * 128:(f + 1) * 128],
                                 xt[:, d, :], start=(d == 0), stop=(d == 7))
            nc.scalar.activation(qt[:, f, qs * CH:(qs + 1) * CH], psq[:], IDN,
                                 bias=env["bias_q"][:, f:f + 1])


def _emit_attn(nc, wk, ps, env, masks, qs, nk, rep, fill=None,
               fill_every=2):
    """Attention for query slot qs (0=lo, 1=hi) over nk kv tiles.

    fill: optional list of zero-arg callables (small PE chain units)
    interleaved into the kv loop every fill_every tiles to keep the PE
    busy through the Act-bound exp stretch."""
    kt, qt, vsb = env["kt"], env["qt"], env["vsb"]
    pending = None  # deferred normalize of previous head pair
    nunit = 0
    for f in range(8):  # head pair (2f, 2f+1)
        pvs = [ps.tile([65, CH], f32, tag=f"pv{e}", bufs=1,
                       name=f"pv{e}_{qs}_{f}_{rep}") for e in range(2)]
        for ki in range(nk):
            st = ps.tile([128, 2 * CH], f32, tag="st", bufs=2,
                         name=f"st{qs}_{f}_{ki}_{rep}")
            for e in range(2):
                nc.tensor.matmul(
                    st[:, e * CH:(e + 1) * CH],
                    kt[64 * e:64 * e + 64, f, ki * 128:(ki + 1) * 128],
                    qt[64 * e:64 * e + 64, f, qs * CH:(qs + 1) * CH],
                    start=True, stop=True, tile_position=(64 * e, 0))
            if ki == 0 and pending is not None:
                _emit_normalize(nc, wk, ps, env, *pending)
                pending = None
            pt = wk.tile([128, 2 * CH], bf16, tag="pt", bufs=3,
                         name=f"pt{qs}_{f}_{ki}_{rep}")
            nc.scalar.activation(pt[:], st[:], EXP, scale=SCALE)
            if qs == 0 or ki >= 8:  # masked tile positions
                eng = [nc.vector, nc.gpsimd]
                mki = ki if qs == 0 else ki - 8
                for e in range(2):
                    eng[e].tensor_mul(pt[:, e * CH:(e + 1) * CH],
                                      pt[:, e * CH:(e + 1) * CH],
                                      masks[:, mki, :])
            for e in range(2):
                nc.tensor.matmul(pvs[e][:], vsb[:, ki, 2 * f + e, :],
                                 pt[:, e * CH:(e + 1) * CH],
                                 start=(ki == 0), stop=(ki == nk - 1))
            nunit += 1
            if fill and nunit % fill_every == 0:
                fill.pop(0)()
        pending = (pvs, qs, f, rep)
    _emit_normalize(nc, wk, ps, env, *pending)
    while fill:
        fill.pop(0)()


def _emit_normalize(nc, wk, ps, env, pvs, qs, f, rep):
    af, zl, rtmp = env["af"], env["zl"], env["rtmp"]
    for e in range(2):
        with nc.allow_low_precision(reason="softmax denominator reciprocal"):
            nc.vector.reciprocal(rtmp[e][64:65, :], pvs[e][64:65, :])
        bcf = ps.tile([128, CH], f32, tag="mm", bufs=2,
                      name=f"bc{qs}_{f}_{e}_{rep}")
        nc.tensor.matmul(bcf[0:64, :], zl[:], rtmp[e][:],
                         start=True, stop=True)
        rb = wk.tile([64, CH], bf16, tag="rb", bufs=2,
                     name=f"rb{qs}_{f}_{e}_{rep}")
        nc.vector.tensor_copy(rb[:], bcf[0:64, :])
        if e == 0:
            nc.vector.tensor_mul(af[0:64, f, qs * CH:(qs + 1) * CH],
                                 pvs[e][0:64, :], rb[:])
        else:
            # odd head lands on partitions 64-127: mul to staging, move
            # across partitions with an identity matmul, copy out of PSUM
            afs = wk.tile([64, CH], bf16, tag="afs", bufs=2,
                          name=f"afs{qs}_{f}_{rep}")
            nc.vector.tensor_mul(afs[:], pvs[e][0:64, :], rb[:])
            mv = ps.tile([128, CH], f32, tag="mm", bufs=2,
                         name=f"mv{qs}_{f}_{rep}")
            nc.tensor.matmul(mv[64:128, :], env["id64"][:], afs[:],
                             start=True, stop=True, tile_position=(0, 64))
            nc.vector.tensor_copy(af[64:128, f, qs * CH:(qs + 1) * CH],
                                  mv[64:128, :])


def _emit_fc_tile(nc, wk, ps, env, wfc, qs, o, rep):
    pfc = ps.tile([128, CH], f32, tag="mm", bufs=2, name=f"pfc{qs}_{o}_{rep}")
    for h in range(8):
        nc.tensor.matmul(pfc[:], wfc[:, h, o, :],
                         env["af"][:, h, qs * CH:(qs + 1) * CH],
                         start=(h == 0), stop=(h == 7))
    ost = wk.tile([128, CH], f32, tag="ost", bufs=2, name=f"ost{qs}_{o}_{rep}")
    nc.vector.tensor_scalar_add(ost[:], pfc[:], env["bias_f"][:, o:o + 1])
    nc.sync.dma_start(env["outT"][o * 128:(o + 1) * 128,
                                  qs * CH:(qs + 1) * CH], ost[:])


def _emit_body(nc, tc, env, rep):
    with tc.tile_pool(name=f"ps_{rep}", bufs=1, space="PSUM") as ps, \
         tc.tile_pool(name=f"at_{rep}", bufs=1) as wk:
        # wk: pt (6KB) + rb (2KB) + mask (8KB) tags, spans attn phases
        with tc.tile_pool(name=f"xt_{rep}", bufs=1) as xp:
            with tc.tile_pool(name=f"w1_{rep}", bufs=1) as w1:
                xt0 = _load_x_chunk(nc, xp, env, 0, rep)
                wkv = w1.tile([128, 8, 2 * D], bf16, name=f"wkv{rep}")
                wsrc = env["wqkv"][:, D:3 * D].rearrange(
                    "(d p) f -> p d f", p=128)
                # K weights in per-ftile pieces so PE starts early
                for f in range(8):
                    nc.gpsimd.dma_start(wkv[:, :, f * 128:(f + 1) * 128],
                                        wsrc[:, :, f * 128:(f + 1) * 128])
                for vh in range(4):
                    nc.gpsimd.dma_start(
                        wkv[:, :, D + vh * 256:D + (vh + 1) * 256],
                        wsrc[:, :, D + vh * 256:D + (vh + 1) * 256])
                masks = wk.tile([128, NKLO, CH], bf16, tag="mask", bufs=1,
                                name=f"mlo{rep}")
                nc.gpsimd.dma_start(masks[:], env["mask_d"][:, 0:NKLO * CH]
                                    .rearrange("p (t c) -> p t c", c=CH))

                # --- segment A: ch0, ch1 KV + all Q ---
                with tc.tile_pool(name=f"wq_{rep}", bufs=1) as wqp:
                    wqq = wqp.tile([128, 8, D], bf16, name=f"wqq{rep}")
                    nc.gpsimd.dma_start(wqq[:], env["wqkv"][:, 0:D].rearrange(
                        "(d p) f -> p d f", p=128))
                    _emit_kv_chunk(nc, xp, ps, env, wkv, 0, rep,
                                   act_copy=True, xt=xt0)
                    _emit_kv_chunk(nc, xp, ps, env, wkv, 1, rep, act_copy=True)
                    _emit_q_chunks(nc, xp, ps, env, wqq, rep)

                # --- segment B: attn-lo || ch2, ch3 KV ---
                xt2 = _load_x_chunk(nc, xp, env, 2, rep)
                xt3 = _load_x_chunk(nc, xp, env, 3, rep)
                fill = (_kv_chunk_units(nc, ps, env, wkv, 2, rep, False, xt2)
                        + _kv_chunk_units(nc, ps, env, wkv, 3, rep, False,
                                          xt3))
                _emit_attn(nc, wk, ps, env, masks, 0, NKLO, rep, fill=fill,
                           fill_every=2)

                masks2 = wk.tile([128, NKLO, CH], bf16, tag="mask", bufs=1,
                                 name=f"mhi{rep}")
                nc.gpsimd.dma_start(masks2[:], env["mask_d"][:, NKLO * CH:]
                                      .rearrange("p (t c) -> p t c", c=CH))

        # --- segment C: attn-hi || FC-lo ---
        with tc.tile_pool(name=f"w2_{rep}", bufs=1) as w2:
            wfc = w2.tile([128, 8, 8, 128], bf16, name=f"wfc{rep}")
            nc.sync.dma_start(wfc[:], env["wfc"][:].rearrange(
                "p (h o m) -> p h o m", h=8, o=8))
            fill = [lambda o=o: _emit_fc_tile(nc, w2, ps, env, wfc, 0, o, rep)
                    for o in range(8)]
            _emit_attn(nc, wk, ps, env, masks2, 1, NKHI, rep, fill=fill,
                       fill_every=14)
            # --- FC-hi ---
            for o in range(8):
                _emit_fc_tile(nc, w2, ps, env, wfc, 1, o, rep)


def _host_inputs(x, W_qkv, b_qkv, W_fc, b_fc):
    import ml_dtypes
    x = np.asarray(x, dtype=np.float32)
    W_qkv = np.asarray(W_qkv, dtype=np.float32)
    b_qkv = np.asarray(b_qkv, dtype=np.float32)
    W_fc = np.asarray(W_fc, dtype=np.float32)
    b_fc = np.asarray(b_fc, dtype=np.float32)

    wqkv = np.ascontiguousarray(W_qkv).astype(ml_dtypes.bfloat16)
    bq = np.ascontiguousarray(b_qkv[0:D].reshape(8, 128).T)
    bk = np.ascontiguousarray(b_qkv[D:2 * D].reshape(8, 128).T)
    bkb = np.ascontiguousarray(b_qkv[None, D:2 * D]).astype(ml_dtypes.bfloat16)
    bv = np.ascontiguousarray(b_qkv[None, 2 * D:3 * D]).astype(ml_dtypes.bfloat16)
    # wfc packed [64, h, o, 128]
    # [128, pair, o, 128]: heads 2f on partitions 0-63, 2f+1 on 64-127
    wfc = np.ascontiguousarray(
        W_fc.reshape(8, 2, 64, 8, 128).transpose(1, 2, 0, 3, 4).reshape(128, -1)
    ).astype(ml_dtypes.bfloat16)
    id64 = np.eye(64, dtype=np.float32).astype(ml_dtypes.bfloat16)
    bfc = np.ascontiguousarray(b_fc.reshape(8, 128).T)
    zl = np.zeros((65, 64), np.float32)
    zl[64, :] = 1.0

    p = np.arange(128)[:, None]
    qq = np.arange(CH)[None, :]

    in_maps = []
    for c in range(NCORES):
        b = c // 2
        v = c % 2
        qb_lo, qb_hi = (0, 3) if v == 0 else (1, 2)
        q0_lo, q0_hi = qb_lo * CH, qb_hi * CH
        xb = x[b]                                   # [T, D]
        xT = np.ascontiguousarray(xb.T).astype(ml_dtypes.bfloat16)
        xq = np.ascontiguousarray(np.concatenate(
            [xb[q0_lo:q0_lo + CH], xb[q0_hi:q0_hi + CH]], axis=0).T
        ).astype(ml_dtypes.bfloat16)
        m = np.zeros((128, NKHI, CH), np.float32)
        for j in range(NKHI):
            q0 = q0_lo if j < 8 else q0_hi
            m[:, j, :] = (128 * j + p <= q0 + qq)
        masks = np.ascontiguousarray(m.reshape(128, -1)).astype(
            ml_dtypes.bfloat16)
        in_maps.append({
            "xT": xT, "xq": xq, "wqkv": wqkv, "bq": bq, "bk": bk, "bkb": bkb,
            "bv": bv, "wfc": wfc, "bfc": bfc, "masks": masks, "zl": zl, "id64": id64,
        })
    return in_maps


def _get_nc():
    if "nc" not in _CACHE:
        _CACHE["nc"] = _build()
    return _CACHE["nc"]


def _assemble(results):
    out = np.empty((B, T, D), np.float32)
    for c in range(NCORES):
        b = c // 2
        v = c % 2
        qb_lo, qb_hi = (0, 3) if v == 0 else (1, 2)
        oT = results[c]["outT"]                     # [D, QLOC]
        out[b, qb_lo * CH:(qb_lo + 1) * CH] = oT[:, 0:CH].T
        out[b, qb_hi * CH:(qb_hi + 1) * CH] = oT[:, CH:2 * CH].T
    return out


def kernel(x, W_qkv, b_qkv, W_fc, b_fc):
    nc = _get_nc()
    in_maps = _host_inputs(x, W_qkv, b_qkv, W_fc, b_fc)
    res = run_bass_kernel_spmd(nc, in_maps, list(range(NCORES)))
    return _assemble(res.results)
